# revision 27
# baseline (speedup 1.0000x reference)
"""Trainium2 Bass kernel for nn_LoRA_QKVlinear (VeRA-style LoRA on K/V of a QKV linear).

Reference computation (fp32):
    delta_k = diag(vera_b[k]) @ vera_B @ diag(vera_d[k]) @ vera_A   for k in {K, V}
    W_eff   = base_weight + concat([0, delta_K, delta_V], axis=0)   # (3072, 1024)
    y       = x @ W_eff.T + base_bias                               # (4, 4096, 3072)

Sharding: data-parallel over tokens (B*S = 16384 -> 2048 per core).  Each of the
8 cores gets the full (replicated) weights + vera tensors and computes the full
3072 output features for its token slice.  No collectives; host concatenates.

Precision: all-fp8 (e4m3) "double-fp8" decomposition.  Both operands are
scaled into e4m3's normal range (x*32, W*1024) and quantized; the residuals
rx8 = Q(u - x8) and rW8 = Q(v - W8) are themselves fp8 and land on the SAME
psum scale, so each 256-row chunk-pair accumulates
    T1 = x8@W8  (+ T3 = rx8@W8)  (+ T2 = x8@rW8)
with no per-term rescaling.  A 3-term pair has ~delta^2 error (better than
bf16) at 3 DoubleRow matmuls; config is [3,3,2,1]: chunk-pairs 0-1 get all
three terms, pair 2 drops its W-residual term, pair 3 runs T1 only.  All
matmuls run as fp8 DoubleRow (0.5 cy/row, 2 k-chunks per instruction): 9 DR
per 512-wide output tile = 2304 PE cycles vs the bf16 baseline's 3328.
Measured scale-relative error 1.693e-2 vs the 2e-2 gate (deterministic
through the CoreSim fallback path).

The VeRA delta (~0.3% of W, so its path has huge error headroom) is computed
on device in fp8 DoubleRow (32 matmuls) and folded by GPSIMD STT ops:
pairs 0-1 write the K/V rW8 tiles from a bf16-staged base residual; pairs
2-3 add the delta into their K/V W8 slices in place (single fp8 rounding).

Device schedule (per core), against the TRN2 timeline cost model:
  - 28 dummy bf16 matmuls burn the PE p-state ramp while the first DMAs
    land, ending just as tile (0,0)'s operands arrive (~4.5us).
  - o-tile-major main loop: per (o-tile, token-tile) a 9-matmul psum chain,
    DVE STT evicts (psum/32768 + bias -> bf16) into token-pair ys buffers,
    one y DMA per 256-token pair (halves the HWDGE descriptor-gen count).
  - x8/rx8 stream in token chunks [128,128,256x7] paced just ahead of
    their token tiles; W-tile 0 is split into column halves and the ot0
    bias slice is DMA'd up front.  vera_B and the K/V base residual are
    staged fp8 (error-neutral, halves two DMAs in the critical window).
  - bbT8_k = f8(bf16(B.T * d_k) * b_k) via ACT per-partition-scale copies +
    DVE broadcast multiplies; the 32 delta groups weave into ot1 (from
    tt3, ramping 1/tt then 2/tt; all 16 K groups complete inside ot1) and
    ot2 (V groups, 1/tt), paced to GPSIMD's STT throughput so delta psum
    banks never back up.
  - Final tile computed as two independent 256-wide psum halves on two DMA
    queues, halving the kernel-end evict->DMA->semaphore chain.

TimelineSim per-core exec: 105.0us (baseline bf16 kernel: 145.6us).
"""

import numpy as np
import ml_dtypes

import concourse.bass as bass
import concourse.mybir as mybir
import concourse.tile as tile
from concourse import bass_utils

# ---------------------------------------------------------------------------
# Workaround: the walrus build in this container caps sync-wait commands per
# instruction, but TileContext's kernel-tail drain carries a wait for every
# logical processor (27), so codegen fails with "Too many sync wait commands"
# for ANY Tile kernel.  Split the tail-drain waits across several drain
# instructions (<=4 waits each, same sync engine => program order preserves
# the barrier semantics).  The epilogue keeps the post-drain engine barrier
# (all work complete, output final in DRAM) but drops the semaphore
# clear-and-free pass + second barrier: this kernel is single-shot per NEFF
# load, so sem cleanup for re-execution is dead time (~0.3us/core).
# ---------------------------------------------------------------------------
from bass_rust import ScopedClock as _ScopedClock, VectorClock as _VectorClock


def _split_drain_and_barrier(self, tick_clock, wait_clock):
    gc = tick_clock.global_clock
    n = len(gc)
    CH = 4
    for s in range(0, n, CH):
        vec = [0] * n
        nz = False
        for i in range(s, min(s + CH, n)):
            vec[i] = gc[i]
            nz = nz or gc[i] > 0
        if not nz:
            continue
        di = self.nc.sync.drain()
        wait_clock.add_sem_waits(di.ins, _ScopedClock({None: _VectorClock(vec)}))

    self.nc.all_engine_barrier()
    assert self.sems is not None
    popped = self.nc._tile_sem_poison_stack.pop()
    assert popped is self._sem_poison


tile.TileContext._drain_and_barrier = _split_drain_and_barrier

N_CORES = 8
B, S = 4, 4096
I = 1024          # in features
O = 1024          # per-projection out features
O3 = 3 * O        # 3072 total out features
R = 256           # vera rank
T_TOTAL = B * S   # 16384 tokens
T = T_TOTAL // N_CORES  # 2048 tokens per core
P = 128
KO = I // P       # 8 contraction chunks
KB = 4            # chunks with a host/device W-residual (pairs 0..1)
IB = KB * P       # 512 in-features covered by rW8
NPAIR = 4         # DoubleRow chunk pairs
RO = R // P       # 2 rank chunks
NT = 512          # output-feature tile (one PSUM bank of fp32)
OT = O3 // NT     # 6 output tiles
TT = T // P       # 16 token tiles
SX = 32.0         # x scale: |x*32| <= ~174 < 240 (e4m3 max)
SW = 1024.0       # W scale: |W*1024| <= ~115
SA = 1024.0       # vera_A scale
SD = 4096.0       # vera_d scale (folded into the staged d vector)
C_OUT = 1.0 / (SX * SW)      # psum -> y descale
C_DELTA = SW / (SA * SD)     # delta psum -> W*SW units
F32 = mybir.dt.float32
BF16 = mybir.dt.bfloat16
FP8 = mybir.dt.float8e4
BF = ml_dtypes.bfloat16
F8 = ml_dtypes.float8_e4m3
DR = mybir.MatmulPerfMode.DoubleRow

# token chunks for x8/rx8 streaming.  All chunks are 512 tokens so every
# DMA keeps >=512B contiguous runs (shorter runs pay a 2x DMA latency
# multiplier); chunk 0 is instead DMA'd in two ko-halves so the first
# matmuls (pairs 0-1) can start after ~1.5us of transfer.
XCHUNKS = [(512 * c, 512) for c in range(4)]


def _build_kernel():
    nc = bass.Bass("TRN2", debug=False, target_bir_lowering=False)

    x8_d = nc.dram_tensor("x8T", [I, T], FP8, kind="ExternalInput")
    rx8_d = nc.dram_tensor("rx8T", [I, T], FP8, kind="ExternalInput")
    w8_d = nc.dram_tensor("w8T", [I, O3], FP8, kind="ExternalInput")
    rw8q_d = nc.dram_tensor("rw8qT", [IB, O], FP8, kind="ExternalInput")
    rbase_d = nc.dram_tensor("rbaseT", [IB, 2 * O], FP8, kind="ExternalInput")
    bias_d = nc.dram_tensor("bias", [O3], BF16, kind="ExternalInput")
    a8_d = nc.dram_tensor("vera_A8", [R, I], FP8, kind="ExternalInput")
    bT8_d = nc.dram_tensor("vera_BT8", [R, O], FP8, kind="ExternalInput")
    d_d = nc.dram_tensor("vera_ds", [2, R], F32, kind="ExternalInput")
    b_d = nc.dram_tensor("vera_b", [2, O], BF16, kind="ExternalInput")
    y_d = nc.dram_tensor("y", [T, O3], BF16, kind="ExternalOutput")

    with tile.TileContext(nc) as tc:
        _kernel_body(tc, x8_d, rx8_d, w8_d, rw8q_d, rbase_d, bias_d, a8_d,
                     bT8_d, d_d, b_d, y_d)
    return nc


def _kernel_body(tc, x8_d, rx8_d, w8_d, rw8q_d, rbase_d, bias_d, a8_d,
                 bT8_d, d_d, b_d, y_d):
    nc = tc.nc
    MUL = mybir.AluOpType.mult
    ADD = mybir.AluOpType.add
    COPY = mybir.ActivationFunctionType.Copy

    x8_r = x8_d.ap().rearrange("(ko p) t -> p ko t", p=P)
    rx8_r = rx8_d.ap().rearrange("(ko p) t -> p ko t", p=P)
    w8_r = w8_d.ap().rearrange("(ko p) o -> p ko o", p=P)
    rw8q_r = rw8q_d.ap().rearrange("(ko p) o -> p ko o", p=P)
    rbase_r = rbase_d.ap().rearrange("(ko p) o -> p ko o", p=P)
    y_r = y_d.ap().rearrange("(tp p) o -> p tp o", p=P)

    with (
        tc.tile_pool(name="persist", bufs=1) as persist,
        tc.tile_pool(name="psum", bufs=8, space="PSUM") as psum_pool,
        tc.tile_pool(name="ypool", bufs=18) as ypool,
    ):
        tt_map = []
        for ci, (cst, clen) in enumerate(XCHUNKS):
            for tj in range(clen // P):
                tt_map.append((ci, tj))
        x8_sb = [persist.tile([P, KO, clen], FP8, name=f"x8_{c}")
                 for c, (_, clen) in enumerate(XCHUNKS)]
        rx8_sb = [persist.tile([P, KO, clen], FP8, name=f"rx8_{c}")
                  for c, (_, clen) in enumerate(XCHUNKS)]
        # rW8 tiles 2..5 (K/V) are device-written by the delta folds.
        w8_sb = [persist.tile([P, KO, NT], FP8, name=f"w8_{j}")
                 for j in range(OT)]
        rw8_sb = [persist.tile([P, KB, NT], FP8, name=f"rw8_{j}")
                  for j in range(OT)]
        rbase_sb = persist.tile([P, KB, 2 * O], FP8)
        bias_sb = persist.tile([P, O3], BF16)
        a8_sb = persist.tile([P, RO, I], FP8)
        bT_sb = persist.tile([P, RO, O], FP8)
        d_sb = persist.tile([P, 2, RO], F32)
        b_bc = persist.tile([P, 2, O], BF16)
        bbt_tmp = [persist.tile([P, RO, O], BF16, name=f"bbt{k}")
                   for k in range(2)]
        bbT8 = [persist.tile([P, RO, O], FP8, name=f"bbT8_{k}")
                for k in range(2)]
        warm_sb = persist.tile([P, P], BF16)

        # ---- PE pre-warm: zero a dummy tile, then issue matmuls on it so
        # the PE p-state ramp (full clock only after ~3us of sustained busy
        # in the cost model) burns off while the first input DMAs are in
        # flight; the real matmuls then start at full rate.
        nc.gpsimd.memset(warm_sb[:], 0.0)
        warm_ps = psum_pool.tile([P, P], F32, tag="ps", name="warm_ps")
        for _ in range(28):
            nc.tensor.matmul(warm_ps[:], warm_sb[:], warm_sb[:],
                             start=True, stop=True)

        # ---- input DMAs, ordered for earliest PE start on the shared DMA
        # resource; chunk 0 and W-tile 0 arrive in ko-halves so the lo-pair
        # matmuls of tile (0,0) start after ~2 transfers, the ot0 bias slice
        # lands before the first eviction, and vera_B/d arrive early enough
        # that the bbT chain beats the first woven delta matmul.
        def dma_x(c):
            cst, clen = XCHUNKS[c]
            nc.sync.dma_start(x8_sb[c][:], x8_r[:, :, cst:cst + clen])
            nc.sync.dma_start(rx8_sb[c][:], rx8_r[:, :, cst:cst + clen])

        nc.sync.dma_start(x8_sb[0][:, 0:KO // 2, :], x8_r[:, 0:KO // 2, 0:NT])
        nc.sync.dma_start(w8_sb[0][:, 0:KO // 2, :], w8_r[:, 0:KO // 2, 0:NT])
        nc.sync.dma_start(rx8_sb[0][:, 0:KO // 2, :], rx8_r[:, 0:KO // 2, 0:NT])
        nc.sync.dma_start(x8_sb[0][:, KO // 2:, :], x8_r[:, KO // 2:KO, 0:NT])
        nc.sync.dma_start(w8_sb[0][:, KO // 2:, :], w8_r[:, KO // 2:KO, 0:NT])
        nc.sync.dma_start(rx8_sb[0][:, KO // 2:KO - 2, :],
                          rx8_r[:, KO // 2:KO - 2, 0:NT])
        nc.sync.dma_start(rw8_sb[0][:], rw8q_r[:, :, 0:NT])
        def dma_x_split(c):
            # ko-halved chunk loads: each piece's completion-sem latency
            # hides under the next piece's transfer.
            cst = XCHUNKS[c][0]
            nc.sync.dma_start(x8_sb[c][:, 0:KO // 2, :],
                              x8_r[:, 0:KO // 2, cst:cst + NT])
            nc.sync.dma_start(rx8_sb[c][:, 0:KO // 2, :],
                              rx8_r[:, 0:KO // 2, cst:cst + NT])
            nc.sync.dma_start(x8_sb[c][:, KO // 2:, :],
                              x8_r[:, KO // 2:KO, cst:cst + NT])
            nc.sync.dma_start(rx8_sb[c][:, KO // 2:KO - 2, :],
                              rx8_r[:, KO // 2:KO - 2, cst:cst + NT])

        dma_x_split(1)
        nc.sync.dma_start(bias_sb[:, 0:NT],
                          bias_d.ap()[0:NT].partition_broadcast(P))
        dma_x_split(2)
        dma_x_split(3)
        nc.sync.dma_start(w8_sb[1][:], w8_r[:, :, NT:2 * NT])
        nc.sync.dma_start(rw8_sb[1][:], rw8q_r[:, :, NT:2 * NT])
        nc.sync.dma_start(bT_sb[:], bT8_d.ap().rearrange("(ro p) o -> p ro o", p=P))
        nc.sync.dma_start(d_sb[:], d_d.ap().rearrange("k (ro p) -> p k ro", p=P))
        nc.sync.dma_start(b_bc[:], b_d.ap().partition_broadcast(P))
        nc.sync.dma_start(a8_sb[:], a8_d.ap().rearrange("(ro p) i -> p ro i", p=P))
        nc.sync.dma_start(bias_sb[:, NT:O3],
                          bias_d.ap()[NT:O3].partition_broadcast(P))
        nc.sync.dma_start(rbase_sb[:, :, 0:O], rbase_r[:, :, 0:O])
        nc.sync.dma_start(w8_sb[2][:], w8_r[:, :, 2 * NT:3 * NT])
        nc.sync.dma_start(rbase_sb[:, :, O:2 * O], rbase_r[:, :, O:2 * O])
        for j in range(3, OT):
            nc.sync.dma_start(w8_sb[j][:], w8_r[:, :, j * NT:(j + 1) * NT])

        def emit_bbT():
            # bbT8_k[r, o] = f8( bf16(B.T[r,o] * d_k[r]) * b_k[o] ):
            # per-partition d scale on ACT, per-column b broadcast on DVE.
            for k in range(2):
                for ro in range(RO):
                    nc.scalar.activation(
                        bbt_tmp[k][:, ro, :], bT_sb[:, ro, :], COPY,
                        scale=d_sb[:, k, ro:ro + 1])
                nc.vector.tensor_tensor(
                    bbT8[k][:], bbt_tmp[k][:],
                    b_bc[:, k, None, :].to_broadcast([P, RO, O]), MUL)

        def mm_chain(ps, ot, tt, o_off=0, o_cw=NT):
            """The 9-matmul accumulation chain for one (o-tile, token-tile):
            T1 (x8@W8) on all 4 pairs, T3 (rx8@W8) on pairs 0-2, T2
            (x8@rW8) on pairs 0-1; all fp8 DoubleRow, one accumulation group.
            Pairs 0-1 (both terms) run before pairs 2-3 so tile (0,0) can
            start on the ko-lo DMA halves alone.  (o_off, o_cw) selects a
            column window for the final-tile halves."""
            ci, tj = tt_map[tt]
            xs, rxs = x8_sb[ci], rx8_sb[ci]
            wt, rwt = w8_sb[ot], rw8_sb[ot]
            tsl = slice(tj * P, (tj + 1) * P)
            ops = ([(xs, wt, p) for p in range(2)]
                   + [(rxs, wt, p) for p in range(2)]
                   + [(xs, wt, p) for p in range(2, NPAIR)]
                   + [(rxs, wt, p) for p in range(2, 3)]
                   + [(xs, rwt, p) for p in range(KB // 2)])
            n = len(ops)
            for idx, (lhs, w, p) in enumerate(ops):
                nc.tensor.matmul(
                    ps[:, 0:o_cw],
                    lhs[:, 2 * p:2 * p + 2, tsl],
                    w[:, 2 * p:2 * p + 2, o_off:o_off + o_cw],
                    start=(idx == 0), stop=(idx == n - 1), perf_mode=DR)

        kgroups = [(0, ic, oth) for ic in range(KO) for oth in range(2)]
        vgroups = [(1, ic, oth) for ic in range(KO) for oth in range(2)]
        groups = kgroups + vgroups

        def delta_group(k, ic, oth):
            """delta.T[i-chunk ic, o-half oth] for projection k (one fp8 DR
            matmul over both rank chunks), folded by GPSIMD: chunks 0..3 add
            the bf16-staged base residual and write the K/V rW8 tile; chunks
            4..7 (the 2-term pairs) add into the K/V W8 slices in place."""
            pd = psum_pool.tile([P, NT], F32, tag="ps", name=f"pd_{k}_{ic}_{oth}")
            nc.tensor.matmul(pd[:], a8_sb[:, :, ic * P:(ic + 1) * P],
                             bbT8[k][:, :, oth * NT:(oth + 1) * NT],
                             start=True, stop=True, perf_mode=DR)
            j = 2 + 2 * k + oth
            if ic < KB:
                nc.gpsimd.scalar_tensor_tensor(
                    rw8_sb[j][:, ic, :], pd[:], C_DELTA,
                    rbase_sb[:, ic, k * O + oth * NT:k * O + (oth + 1) * NT],
                    MUL, ADD)
            else:
                nc.gpsimd.scalar_tensor_tensor(
                    w8_sb[j][:, ic, :], pd[:], C_DELTA, w8_sb[j][:, ic, :],
                    MUL, ADD)

        # delta-group weave: 2/tt over ot1 tt4-11, 1/tt over ot1 tt12-15 and
        # ot2 tt0-11 -- paced to GPSIMD STT throughput (~0.8us each) so delta
        # psum banks never back up, every fold lands before its W tile is
        # read (K tiles at ot2, V tiles at ot4), and the bbT chain + rbase
        # DMA have until ot1-tt4 to finish.
        weave = {}
        gi = 0
        sched = ([(1, tt, 1) for tt in range(3, 10)]
                 + [(1, tt, 2) for tt in range(10, 14)]
                 + [(1, tt, 1) for tt in range(14, 16)]
                 + [(2, tt, 1) for tt in range(15)])
        for (ot, tt, ng) in sched:
            weave[(ot, tt)] = groups[gi:gi + ng]
            gi += ng
        assert gi == 32

        def phase(ot):
            for tp in range(TT // 2):
                if ot == OT - 1 and tp == TT // 2 - 1:
                    _final_pair()
                    return
                ys = ypool.tile([P, 2, NT], BF16, tag="ys", name=f"ys_{ot}_{tp}")
                for sub in range(2):
                    tt = 2 * tp + sub
                    ps = psum_pool.tile([P, NT], F32, tag="ps",
                                        name=f"ps_{ot}_{tt}")
                    mm_chain(ps, ot, tt)
                    nc.vector.scalar_tensor_tensor(
                        ys[:, sub, :], ps[:], C_OUT,
                        bias_sb[:, ot * NT:(ot + 1) * NT], MUL, ADD)
                    if ot == 0 and tt == 1:
                        emit_bbT()
                    for g in weave.get((ot, tt), ()):
                        delta_group(*g)
                nc.sync.dma_start(
                    y_r[:, 2 * tp:2 * tp + 2, ot * NT:(ot + 1) * NT], ys[:])

        def _final_pair():
            # tt14: its own single-tile DMA; tt15: two independent 256-wide
            # psum halves so the first half's evict + DMA drain during the
            # second half's matmuls, halving the kernel-end chain.
            ot = OT - 1
            ys14 = ypool.tile([P, NT], BF16, tag="ys", name="ys14")
            ps14 = psum_pool.tile([P, NT], F32, tag="ps", name="ps14")
            mm_chain(ps14, ot, TT - 2)
            nc.vector.scalar_tensor_tensor(
                ys14[:], ps14[:], C_OUT, bias_sb[:, ot * NT:(ot + 1) * NT],
                MUL, ADD)
            nc.sync.dma_start(y_r[:, TT - 2, ot * NT:(ot + 1) * NT], ys14[:])
            # the final tile runs as two independent psum windows on two
            # DMA queues, so the first window's evict + write-back chain
            # drains during the second window's matmuls.
            for h, (o_off, o_cw, eng) in enumerate(
                    [(0, NT // 2, nc.gpsimd), (NT // 2, NT // 2, nc.scalar)]):
                psh = psum_pool.tile([P, o_cw], F32, tag="ps",
                                     name=f"ps_last_{h}")
                mm_chain(psh, ot, TT - 1, o_off=o_off, o_cw=o_cw)
                ysh = ypool.tile([P, o_cw], BF16, tag="ys",
                                 name=f"ys_last_{h}")
                nc.vector.scalar_tensor_tensor(
                    ysh[:], psh[:], C_OUT,
                    bias_sb[:, ot * NT + o_off:ot * NT + o_off + o_cw],
                    MUL, ADD)
                eng.dma_start(
                    y_r[:, TT - 1, ot * NT + o_off:ot * NT + o_off + o_cw],
                    ysh[:])

        for ot in range(OT):
            phase(ot)


_cached_nc = None


def _get_nc():
    global _cached_nc
    if _cached_nc is None:
        _cached_nc = _build_kernel()
    return _cached_nc


def _make_in_maps(x, base_weight, base_bias, vera_A, vera_B, vera_d, vera_b):
    x2 = np.asarray(x, dtype=np.float32).reshape(T_TOTAL, I)
    u = x2 * SX
    x8 = u.astype(F8)
    rx8 = (u - x8.astype(np.float32)).astype(F8)
    v = np.asarray(base_weight, dtype=np.float32) * SW
    W8 = v.astype(F8)
    rW = v - W8.astype(np.float32)
    w8T = np.ascontiguousarray(W8.T)                              # (1024, 3072)
    rw8qT = np.ascontiguousarray(rW[:O, :IB].T.astype(F8))        # (768, 1024)
    rbaseT = np.ascontiguousarray(rW[O:, :IB].T.astype(F8))       # (512, 2048)
    bias = np.ascontiguousarray(np.asarray(base_bias, dtype=np.float32).astype(BF))
    a8 = np.ascontiguousarray((np.asarray(vera_A, dtype=np.float32) * SA).astype(F8))
    bT8 = np.ascontiguousarray(np.asarray(vera_B, dtype=np.float32).T.astype(F8))
    ds = np.ascontiguousarray(np.asarray(vera_d, dtype=np.float32) * SD)
    bv = np.ascontiguousarray(np.asarray(vera_b, dtype=np.float32).astype(BF))
    in_maps = []
    for c in range(N_CORES):
        sl = slice(c * T, (c + 1) * T)
        in_maps.append({
            "x8T": np.ascontiguousarray(x8[sl].T),
            "rx8T": np.ascontiguousarray(rx8[sl].T),
            "w8T": w8T, "rw8qT": rw8qT, "rbaseT": rbaseT, "bias": bias,
            "vera_A8": a8, "vera_BT8": bT8, "vera_ds": ds, "vera_b": bv,
        })
    return in_maps


def _run_coresim(nc, in_maps):
    """Fallback: interpret the BIR per core (bit-accurate, no hardware)."""
    from concourse.bass_interp import CoreSim

    shards = []
    for in_map in in_maps:
        sim = CoreSim(nc, trace=False)
        for name, val in in_map.items():
            sim.tensor(name)[:] = val
        sim.simulate(check_with_hw=False)
        shards.append(np.array(sim.tensor("y")))
    return shards


def kernel(x, base_weight, base_bias, vera_A, vera_B, vera_d, vera_b):
    nc = _get_nc()
    in_maps = _make_in_maps(x, base_weight, base_bias, vera_A, vera_B,
                            vera_d, vera_b)
    try:
        res = bass_utils.run_bass_kernel_spmd(nc, in_maps,
                                              core_ids=list(range(N_CORES)))
        shards = [np.asarray(res.results[c]["y"]) for c in range(N_CORES)]
    except Exception:
        # The axon PJRT execute path can be unavailable in some containers;
        # fall back to interpreting the same BIR so results stay correct.
        shards = _run_coresim(nc, in_maps)
    y = np.concatenate(shards, axis=0).astype(np.float32)
    return y.reshape(B, S, O3)
